# revision 1
# baseline (speedup 1.0000x reference)
"""Trainium2 Bass kernel for nn_MultiHeadAttention_60816736911814.

Reference semantics (all derived from `src`; `k`/`v` args ignored):
  x  = channel_shuffle(src)          # [B,S,G,C]->[B,S,C,G] flatten, G=5
  xh = split_heads(x)                # [B,H,S,dk], H=16, dk=80
  q/k/v = per-head Linear(dk,dk)     # weights [H,dk,dk] + bias
  attn  = softmax(q kᵀ / sqrt(dk)) v
  out   = concat(attn) @ Woᵀ + bo    # Wo [D,D], D=1280

Sharding (8 cores, no collectives): core i handles batch b=i//2 and query
rows [512*(i%2), 512*(i%2)+512). Each core gets src[b] ROLLED so its query
rows are rows 0..511 (key order is irrelevant to softmax+sum), letting all
cores run an identical program. Wo is applied per-core on its row slice, so
the full output is a pure concatenation.

All matmuls run in bf16 with fp32 PSUM accumulation. The channel shuffle,
head split and Linear biases are folded into host-side weight layout:
 - device-side xhT rows use d' ordering with d = 5*(d'%16) + d'//16, so the
   channels of head h at row d' are exactly src channel 256*(d'//16)+16h+
   (d'%16) -> a contiguous 16-channel strip per (h, r=d'//16), produced by
   plain 128x128 PE transposes of src + one rectangular SBUF->SBUF DMA.
 - projection weights are permuted with the same d' order and get the bias
   appended as contraction row 80 (paired with a ones row 80 in xhT).
 - softmax denominator Z comes free as row 80 of the attention matmul by
   augmenting V with a ones column.
"""

import numpy as np
import ml_dtypes

B, S, D = 4, 1024, 1280
H, DK, G = 16, 80, 5
N_CORES = 8
SH = S // 2  # 512 query rows per core
SCALE = 1.0 / float(np.sqrt(DK))
NT = S // 128  # 8 s-tiles
NCT = D // 128  # 10 channel tiles

_BUILT = {}


def _legalize_waits(nc, mybir):
    """This walrus build allows 1 sync-wait per instruction (2 on
    EventSemaphore). Tile can emit more; split overflow waits onto
    injected same-engine NoOp carriers placed just before the
    instruction (engines run their stream in order -> AND semantics)."""
    n_fix = 0
    for f in nc.m.functions:
        for blk in f.blocks:
            out = []
            changed = False
            for inst in blk.instructions:
                cap = 2 if type(inst).__name__ == "InstEventSemaphore" else 1
                si = inst.sync_info
                if si is not None and si.on_wait and len(si.on_wait) > cap:
                    waits = list(si.on_wait)
                    for w in waits[:-cap]:
                        nop = mybir.InstNoOp(name=f"I-waitfix-{n_fix}")
                        n_fix += 1
                        nop.engine = inst.engine
                        nop.sync_info = mybir.SyncInfo(on_wait=[w], on_update=[])
                        out.append(nop)
                    inst.sync_info = mybir.SyncInfo(
                        on_wait=waits[-cap:], on_update=list(si.on_update)
                    )
                    changed = True
                out.append(inst)
            if changed:
                try:
                    blk.instructions = out
                except Exception:
                    blk.instructions.clear()
                    blk.instructions.extend(out)
    return n_fix


def _build(legalize=True):
    import concourse.bass as bass
    import concourse.mybir as mybir
    import concourse.tile as tile

    f32 = mybir.dt.float32
    bf16 = mybir.dt.bfloat16

    nc = bass.Bass(trn_type="TRN2", target_bir_lowering=False, debug=False)

    x_d = nc.dram_tensor("x", [S, D], bf16, kind="ExternalInput").ap()
    wq_d = nc.dram_tensor("wq", [DK + 1, H, DK], bf16, kind="ExternalInput").ap()
    wk_d = nc.dram_tensor("wk", [DK + 1, H, DK], bf16, kind="ExternalInput").ap()
    wv_d = nc.dram_tensor("wv", [DK + 1, H, DK], bf16, kind="ExternalInput").ap()
    wo_d = nc.dram_tensor("wo", [128, 11, D], bf16, kind="ExternalInput").ap()
    idf_d = nc.dram_tensor("identf", [128, 128], bf16, kind="ExternalInput").ap()
    on2_d = nc.dram_tensor("ones2d", [128, 128], bf16, kind="ExternalInput").ap()
    onr_d = nc.dram_tensor("onesrow", [1, H * S], bf16, kind="ExternalInput").ap()
    on80_d = nc.dram_tensor("ones80", [1, DK], bf16, kind="ExternalInput").ap()
    out_d = nc.dram_tensor("out", [SH, D], f32, kind="ExternalOutput").ap()

    with tile.TileContext(nc) as tc:
        with (
            tc.tile_pool(name="const", bufs=1) as const,
            tc.tile_pool(name="big", bufs=1) as big,
            tc.tile_pool(name="ld", bufs=3) as ld,
            tc.tile_pool(name="et", bufs=8) as etp,
            tc.tile_pool(name="sm", bufs=3) as sm,
            tc.tile_pool(name="ps", bufs=4, space="PSUM") as ps,
        ):
            identf = const.tile([128, 128], bf16)
            nc.scalar.dma_start(out=identf, in_=idf_d)
            on2_sb = const.tile([128, 128], bf16)
            nc.scalar.dma_start(out=on2_sb, in_=on2_d)

            wq_sb = big.tile([DK + 1, H, DK], bf16)
            wk_sb = big.tile([DK + 1, H, DK], bf16)
            wv_sb = big.tile([DK + 1, H, DK], bf16)
            wo_sb = big.tile([128, 11, D], bf16)
            nc.scalar.dma_start(out=wq_sb, in_=wq_d)
            nc.scalar.dma_start(out=wk_sb, in_=wk_d)
            nc.scalar.dma_start(out=wv_sb, in_=wv_d)
            nc.scalar.dma_start(out=wo_sb, in_=wo_d)

            # XH[d', h, s]: transposed shuffled heads (+ ones row 80)
            xh = big.tile([DK + 1, H, S], bf16)
            nc.scalar.dma_start(out=xh[DK : DK + 1, :, :], in_=onr_d)
            VW = 97  # Z lands on PSUM partition 96 (32-aligned for engine reads)
            # concatT[e, h, q] and K=128-packed ctp[j%128, j//128, q]
            ct = big.tile([DK + 1, H, SH], bf16)
            ctp = big.tile([128, 11, SH], bf16)
            nc.scalar.dma_start(out=ctp[0:1, 10, :], in_=onr_d[:, 0:SH])

            # ---- Stage 1: load src, cast, transpose (c-outer), repack ----
            xt = big.tile([128, NCT, S], bf16)  # x transposed [c, ct, s]
            sbs = []
            for t in range(NT):
                s_f = ld.tile([128, D], bf16, tag="sf", bufs=NT)
                nc.sync.dma_start(out=s_f, in_=x_d[t * 128 : (t + 1) * 128, :])
                sbs.append(s_f)
            rep = 0
            for c in [0, 2, 4, 6, 8, 1, 3, 5, 7, 9]:
                for t in range(NT):
                    p_ps = ps.tile([128, 128], bf16, tag="rot", bufs=2)
                    nc.tensor.transpose(p_ps, sbs[t][:, c * 128 : (c + 1) * 128], identf)
                    nc.vector.tensor_copy(xt[:, c, t * 128 : (t + 1) * 128], p_ps)
                r = c // 2
                eng = [nc.gpsimd, nc.sync, nc.scalar][rep % 3]
                rep += 1
                for h in range(8 * (c % 2), 8 * (c % 2) + 8):
                    poff = 16 * (h % 8)
                    eng.dma_start(
                        out=xh[16 * r : 16 * r + 16, h, :],
                        in_=xt[poff : poff + 16, c, :],
                    )

            # ---- Stage 4: projections + attention per head ----
            grp_state = {"zg": None, "pend": []}

            def _normalize_group(grp_state=grp_state):
                zg = grp_state["zg"]
                ng = len(grp_state["pend"])
                zr = sm.tile([128, SH], f32, tag="zr", bufs=1, name="zr")
                nc.vector.reciprocal(zr, zg)
                zrb = sm.tile([128, SH], bf16, tag="zrb", bufs=2, name="zrb")
                nc.vector.tensor_copy(zrb, zr)
                for k, (hh, hu) in enumerate(grp_state["pend"]):
                    br_ps = ps.tile(
                        [DK, SH], f32, tag="br", bufs=1, name="br_ps"
                    )
                    nc.tensor.matmul(
                        br_ps, on2_sb[32 * k : 32 * k + 1, 0:DK],
                        zrb[32 * k : 32 * k + 1, :],
                        start=True, stop=True,
                        tile_position=(32 * k, 0),
                    )
                    nc.vector.tensor_mul(ct[0:DK, hh, :], hu, br_ps)
                    j0 = DK * hh
                    pl, off = j0 // 128, j0 % 128
                    l1 = min(128 - off, DK)
                    nc.gpsimd.dma_start(
                        out=ctp[off : off + l1, pl, :], in_=ct[0:l1, hh, :]
                    )
                    if l1 < DK:
                        nc.sync.dma_start(
                            out=ctp[0 : DK - l1, pl + 1, :], in_=ct[l1:DK, hh, :]
                        )
                grp_state["zg"] = None
                grp_state["pend"] = []

            for h in range(H):
                vh = sm.tile([128, NT, VW], bf16, tag="vh", bufs=3, name="vh")
                nc.gpsimd.memset(vh[:, :, DK:VW], 1.0)
                for half in range(2):
                    vp = ps.tile(
                        [128, NT // 2, DK], f32, tag="vp", bufs=2, name="vp"
                    )
                    for u in range(NT // 2):
                        t = half * (NT // 2) + u
                        nc.tensor.matmul(
                            vp[:, u, :],
                            xh[:, h, t * 128 : (t + 1) * 128],
                            wv_sb[:, h, :],
                            start=True,
                            stop=True,
                        )
                    nc.vector.tensor_copy(
                        vh[:, half * (NT // 2) : (half + 1) * (NT // 2), 0:DK], vp
                    )

                qt_ps = ps.tile([DK, SH], f32, tag="qk", bufs=1)
                nc.tensor.matmul(
                    qt_ps, wq_sb[:, h, :], xh[:, h, 0:SH], start=True, stop=True
                )
                qt_sb = sm.tile([DK, SH], bf16, tag="qt", bufs=2)
                nc.vector.tensor_copy(qt_sb, qt_ps)
                kt_sb = sm.tile([DK, S], bf16, tag="kt", bufs=2)
                for j in range(2):
                    kt_ps = ps.tile([DK, SH], f32, tag="qk", bufs=1)
                    nc.tensor.matmul(
                        kt_ps,
                        wk_sb[:, h, :],
                        xh[:, h, j * SH : (j + 1) * SH],
                        start=True,
                        stop=True,
                    )
                    nc.vector.tensor_copy(kt_sb[:, j * SH : (j + 1) * SH], kt_ps)

                hz_ps = ps.tile([VW, SH], f32, tag="hz", bufs=2)
                ets = []
                for t in range(NT):
                    sc_ps = ps.tile([128, SH], f32, tag="rot", bufs=2)
                    nc.tensor.matmul(
                        sc_ps,
                        kt_sb[:, t * 128 : (t + 1) * 128],
                        qt_sb,
                        start=True,
                        stop=True,
                    )
                    et = etp.tile([128, SH], bf16, tag="et")
                    nc.scalar.activation(
                        et, sc_ps, mybir.ActivationFunctionType.Exp, scale=SCALE
                    )
                    ets.append(et)
                for t in range(NT):
                    nc.tensor.matmul(
                        hz_ps,
                        vh[:, t, :],
                        ets[t],
                        start=(t == 0),
                        stop=(t == NT - 1),
                    )
                if grp_state["zg"] is None:
                    grp_state["zg"] = sm.tile(
                        [128, SH], f32, tag="zg", bufs=1, name="zg"
                    )
                k = len(grp_state["pend"])
                nc.scalar.copy(
                    grp_state["zg"][32 * k : 32 * k + 1, :], hz_ps[VW - 1 : VW, :]
                )
                hu = sm.tile([DK, SH], bf16, tag="hu", bufs=4, name="hu")
                nc.vector.tensor_copy(hu, hz_ps[0:DK, :])
                grp_state["pend"].append((h, hu))
                if len(grp_state["pend"]) == 4:
                    _normalize_group()

            # ---- Stage 5: output projection ----
            ocuts = [(0, 512), (512, 1024), (1024, 1280)]
            for qt in range(SH // 128):
                for o0, o1 in ocuts:
                    op = ps.tile([128, 512], f32, tag="rot", bufs=2)
                    for jt in range(11):
                        kh = 1 if jt == 10 else 128
                        nc.tensor.matmul(
                            op[:, 0 : o1 - o0],
                            ctp[0:kh, jt, qt * 128 : (qt + 1) * 128],
                            wo_sb[0:kh, jt, o0:o1],
                            start=(jt == 0),
                            stop=(jt == 10),
                        )
                    o_sb = sm.tile([128, 512], f32, tag="osb", bufs=2)
                    nc.vector.tensor_copy(o_sb[:, 0 : o1 - o0], op[:, 0 : o1 - o0])
                    nc.gpsimd.dma_start(
                        out=out_d[qt * 128 : (qt + 1) * 128, o0:o1],
                        in_=o_sb[:, 0 : o1 - o0],
                    )

    if legalize:
        _legalize_waits(nc, mybir)
    return nc


def _host_prep(Wq, bq, Wk, bk, Wv, bv, Wo, bo):
    bf = ml_dtypes.bfloat16
    dprime = np.arange(DK)
    perm = 5 * (dprime % 16) + dprime // 16  # d' -> d

    def aug(Wx, bx):
        # [H, e, d] -> [H, d', e] permuted, + bias row -> [dk+1, H, dk]
        wt = Wx.transpose(0, 2, 1)[:, perm, :]  # [H, d', e]
        a = np.concatenate([wt, bx[:, None, :]], axis=1)  # [H, dk+1, dk]
        return np.ascontiguousarray(a.transpose(1, 0, 2)).astype(bf)

    wq = aug(Wq, bq)
    wk = aug(Wk, bk)
    wv = aug(Wv, bv)

    wo_t = np.concatenate([Wo.T, np.zeros((128 * 11 - D, D), np.float32)])
    wo_t[D] = bo  # row 0 of plane 10, paired with the ones row in ctp
    wo = np.ascontiguousarray(
        wo_t.reshape(11, 128, D).transpose(1, 0, 2)
    ).astype(bf)

    consts = {
        "identf": np.eye(128, dtype=bf),
        "ones2d": np.ones((128, 128), bf),
        "onesrow": np.ones((1, H * S), bf),
        "ones80": np.ones((1, DK), bf),
    }
    return wq, wk, wv, wo, consts


def kernel(**inputs):
    from concourse.bass_utils import run_bass_kernel_spmd

    src = np.asarray(inputs["src"], np.float32)
    wq, wk, wv, wo, consts = _host_prep(
        np.asarray(inputs["Wq"], np.float32),
        np.asarray(inputs["bq"], np.float32),
        np.asarray(inputs["Wk"], np.float32),
        np.asarray(inputs["bk"], np.float32),
        np.asarray(inputs["Wv"], np.float32),
        np.asarray(inputs["bv"], np.float32),
        np.asarray(inputs["Wo"], np.float32),
        np.asarray(inputs["bo"], np.float32),
    )

    if "nc" not in _BUILT:
        _BUILT["nc"] = _build()
    nc = _BUILT["nc"]

    in_maps = []
    for i in range(N_CORES):
        b, qlo = i // 2, (i % 2) * SH
        x = np.roll(src[b], -qlo, axis=0)
        in_maps.append(
            {
                "x": np.ascontiguousarray(x).astype(ml_dtypes.bfloat16),
                "wq": wq,
                "wk": wk,
                "wv": wv,
                "wo": wo,
                **consts,
            }
        )

    res = run_bass_kernel_spmd(nc, in_maps, core_ids=list(range(N_CORES)))

    out = np.empty((B, S, D), np.float32)
    for i in range(N_CORES):
        b, qlo = i // 2, (i % 2) * SH
        out[b, qlo : qlo + SH] = res.results[i]["out"]
    return out



# revision 30
# speedup vs baseline: 1.2144x; 1.2144x over previous
"""Trainium2 Bass kernel for nn_MultiHeadAttention_60816736911814.

Reference semantics (all derived from `src`; `k`/`v` args ignored):
  x  = channel_shuffle(src)          # [B,S,G,C]->[B,S,C,G] flatten, G=5
  xh = split_heads(x)                # [B,H,S,dk], H=16, dk=80
  q/k/v = per-head Linear(dk,dk)     # weights [H,dk,dk] + bias
  attn  = softmax(q k^T / sqrt(dk)) v
  out   = concat(attn) @ Wo^T + bo   # Wo [D,D], D=1280

Sharding (8 cores, no collectives): core i handles batch b=i//2 and query
rows [512*(i%2), +512). Each core gets src[b] ROLLED so its query rows are
rows 0..511 (key order is irrelevant to softmax+sum); all cores run an
identical program and the full output is a pure concatenation.

Device-side algebraic restructuring (all folds are host-side, layout-only
or tiny weight-matrix products):
 - K projection eliminated: scores = x~_k^T Mt x~_q with Mt = Wq_aug @
   Wk_aug^T per head (x~ = [x;1] handles both biases). One projection
   (qq = Mt^T x~_q) replaces q AND k projections.
 - V projection and output projection fused: out = sum_h Rn_h^T G_h where
   R_h = XS_h P_h ([1+dk, SH], row 0 = Z = softmax denominator via the
   ones-column of XS), Rn = R * bcast(1/Z), and G_h = Wv_aug_h @
   Wo_h_block^T (+ bo folded into head 0's row 0, whose Rn value is 1).
 - Channel shuffle / head split / transposes are folded into the host-side
   layouts of xh (d-major) and xs (s-major): no on-device transposes.

Engine schedule: ACT does only the 48 exp instructions (scores land in
2/3-bank fused PSUM tiles so each Exp covers 1536/1024 columns); casts run
on GpSimd, normalization mul + reciprocal on DVE, 1/Z partition-broadcast
via SBUF->SBUF DMA. The PE stream is software-pipelined (scores(h) ->
qq(h+1) -> R(h-1)) so the Tensor engine stays continuously busy (max
p-state clock); qq and R share one rotating 2-slot PSUM tag so everything
fits the 8 PSUM banks alongside the 6 score banks.
"""

import numpy as np
import ml_dtypes

B, S, D = 4, 1024, 1280
H, DK, G = 16, 80, 5
N_CORES = 8
SH = S // 2  # 512 query rows per core
SCALE = 1.0 / float(np.sqrt(DK))
NT = S // 128  # 8 k-tiles
DA = DK + 1  # 81: augmented channel dim
NJ = H * DA  # 1296 packed (h,d) rows
NPL = (NJ + 127) // 128  # 11 planes
# per-head score fusion: k-tiles per fused PSUM tile / exp instruction
FUSE = [3, 3, 2]

_BUILT = {}


def _legalize_waits(nc, mybir):
    """This walrus build allows 1 sync-wait per instruction (2 on
    EventSemaphore). Tile can emit more; split overflow waits onto
    injected same-engine NoOp carriers placed just before the
    instruction (engines run their stream in order -> AND semantics)."""
    n_fix = 0
    for f in nc.m.functions:
        for blk in f.blocks:
            out = []
            changed = False
            for inst in blk.instructions:
                cap = 2 if type(inst).__name__ == "InstEventSemaphore" else 1
                si = inst.sync_info
                if si is not None and si.on_wait and len(si.on_wait) > cap:
                    waits = list(si.on_wait)
                    for w in waits[:-cap]:
                        nop = mybir.InstNoOp(name=f"I-waitfix-{n_fix}")
                        n_fix += 1
                        nop.engine = inst.engine
                        nop.sync_info = mybir.SyncInfo(on_wait=[w], on_update=[])
                        out.append(nop)
                    inst.sync_info = mybir.SyncInfo(
                        on_wait=waits[-cap:], on_update=list(si.on_update)
                    )
                    changed = True
                out.append(inst)
            if changed:
                try:
                    blk.instructions = out
                except Exception:
                    blk.instructions.clear()
                    blk.instructions.extend(out)
    return n_fix


def _build(legalize=True):
    import os as _os
    import concourse.bass as bass
    import concourse.mybir as mybir
    import concourse.tile as tile

    _fuse = FUSE

    f32 = mybir.dt.float32
    bf16 = mybir.dt.bfloat16

    nc = bass.Bass(trn_type="TRN2", target_bir_lowering=False, debug=False)

    # xh[d, h, s]: d-major shuffled x, row 80 = ones
    xh_d = nc.dram_tensor("xh", [DA, H, S], bf16, kind="ExternalInput").ap()
    # xs[p, t, h, j]: s-major shuffled x (k = t*128+p), col j=0 = ones
    xs_d = nc.dram_tensor("xs", [128, NT, H, DA], bf16, kind="ExternalInput").ap()
    # mt[d1, h, d2] = (Wq_aug @ Wk_aug^T)[d1, d2] per head
    mt_d = nc.dram_tensor("mt", [DA, H, DA], bf16, kind="ExternalInput").ap()
    # g[p, pl, o]: packed rows j = 81h + d of G_h = Wv_aug @ Wo_h^T (+bo)
    g_d = nc.dram_tensor("g", [128, NPL, D], bf16, kind="ExternalInput").ap()
    out_d = nc.dram_tensor("out", [SH, D], f32, kind="ExternalOutput").ap()

    with tile.TileContext(nc) as tc:
        with (
            tc.tile_pool(name="const", bufs=1) as const,
            tc.tile_pool(name="big", bufs=1) as big,
            tc.tile_pool(name="et", bufs=6) as etp,
            tc.tile_pool(name="sm", bufs=2) as sm,
            tc.tile_pool(name="ps", bufs=2, space="PSUM") as ps,
        ):
            on2 = const.tile([128, 512], bf16)
            nc.gpsimd.memset(on2, 1.0)

            mt_sb = big.tile([DA, H, DA], bf16)
            xh_sb = big.tile([DA, H, S], bf16)
            xs_sb = big.tile([128, NT, H, DA], bf16)
            g_sb = big.tile([128, NPL, D], bf16)
            ct = big.tile([DA, H, SH], bf16)  # normalized heads, h-major
            ctp = big.tile([128, NPL, SH], bf16)  # packed rows j = 81h+d

            # --- input DMA: ACT issues none (DMA transfer time occupies the
            # issuing engine, and ACT paces the head phase) ---
            nc.sync.dma_start(out=mt_sb, in_=mt_d)
            nc.sync.dma_start(out=xh_sb[:, 0:2, :], in_=xh_d[:, 0:2, :])
            nc.sync.dma_start(out=xh_sb[:, 2:4, :], in_=xh_d[:, 2:4, :])
            nc.sync.dma_start(out=xh_sb[:, 4:8, :], in_=xh_d[:, 4:8, :])
            nc.gpsimd.dma_start(out=xs_sb[:, 0:4, :, :], in_=xs_d[:, 0:4, :, :])
            nc.sync.dma_start(out=xs_sb[:, 4:8, :, :], in_=xs_d[:, 4:8, :, :])
            nc.sync.dma_start(out=xh_sb[:, 8:16, :], in_=xh_d[:, 8:16, :])
            nc.gpsimd.dma_start(out=g_sb[:, 0:6, :], in_=g_d[:, 0:6, :])
            nc.gpsimd.dma_start(out=g_sb[:, 6:11, :], in_=g_d[:, 6:11, :])

            # --- PE p-state preheat: dummy matmuls so the clock is ramped
            # when real work starts ---
            for i in range(7):
                ph = ps.tile([DA, 512], f32, tag="qr", bufs=2, name="ph")
                nc.tensor.matmul(
                    ph[0:1, :], on2[0:1, 0:1], on2[0:1, :], start=True, stop=True
                )

            qq_sbs = {}
            ets = {}
            r_pss = {}
            hus = {}
            # head groups for softmax normalization; the last groups are
            # small so the final normalization chains are short
            HGROUPS = [
                (0, 1, 2, 3), (4, 5, 6, 7), (8, 9, 10, 11), (12, 13), (14,), (15,),
            ]
            GOF = {h: (gi, k) for gi, g in enumerate(HGROUPS) for k, h in enumerate(g)}
            zgs = {}
            norm_st = {}

            def issue_qq(h):
                qq_ps = ps.tile([DA, 512], f32, tag="qr", bufs=2, name="qq_ps")
                nc.tensor.matmul(
                    qq_ps, mt_sb[:, h, :], xh_sb[:, h, 0:SH], start=True, stop=True
                )
                qq_sb = sm.tile([DA, 512], bf16, tag="qq_sb", bufs=2, name="qq_sb")
                nc.vector.tensor_copy(qq_sb, qq_ps)
                qq_sbs[h] = qq_sb

            def issue_scores(h):
                qq_sb = qq_sbs.pop(h)
                ets[h] = []
                kt = 0
                for nf in _fuse:
                    sc = ps.tile([128, nf * 512], f32, tag="sc", bufs=2, name="sc")
                    for u in range(nf):
                        nc.tensor.matmul(
                            sc[:, u * 512 : u * 512 + 512],
                            xh_sb[:, h, (kt + u) * 128 : (kt + u) * 128 + 128],
                            qq_sb,
                            start=True,
                            stop=True,
                        )
                    kt += nf
                    et = etp.tile(
                        [128, nf * 512], bf16, tag="et", name="et",
                        bufs=3 * len(_fuse),
                    )
                    nc.scalar.activation(
                        et[:, 0 : nf * 512],
                        sc[:, 0 : nf * 512],
                        mybir.ActivationFunctionType.Exp,
                        scale=SCALE,
                    )
                    ets[h].append(et)

            def issue_r(h):
                r_ps = ps.tile([DA, 512], f32, tag="qr", bufs=2, name="r_ps")
                kt = 0
                for f, nf in enumerate(_fuse):
                    for u in range(nf):
                        nc.tensor.matmul(
                            r_ps,
                            xs_sb[:, kt + u, h, :],
                            ets[h][f][:, u * 512 : u * 512 + 512],
                            start=(kt + u == 0),
                            stop=(kt + u == NT - 1),
                        )
                    kt += nf
                del ets[h]
                gi, k = GOF[h]
                if len(HGROUPS[gi]) > 1:
                    if k == 0:
                        zgs[gi] = sm.tile(
                            [128, 512], f32, tag="zg", bufs=2, name="zg"
                        )
                        nc.gpsimd.memset(zgs[gi], 1.0)
                    nc.vector.tensor_copy(
                        zgs[gi][32 * k : 32 * k + 1, :], r_ps[0:1, :]
                    )
                else:
                    r_pss[h] = r_ps  # single-head group: recip reads PSUM row
                hu = sm.tile([DA, 512], bf16, tag="hu", bufs=8, name="hu")
                nc.vector.tensor_copy(hu, r_ps)
                hus[h] = hu

            def norm_a(gi):
                """Reciprocal of Z + broadcast DMAs for group gi."""
                zr = sm.tile([128, 512], f32, tag="zr", bufs=2, name="zr")
                if len(HGROUPS[gi]) > 1:
                    nc.vector.reciprocal(zr, zgs.pop(gi))
                else:
                    h = HGROUPS[gi][0]
                    nc.vector.reciprocal(zr[0:1, :], r_pss.pop(h)[0:1, :])
                zrb = sm.tile([128, 512], bf16, tag="zrb", bufs=2, name="zrb")
                nc.vector.tensor_copy(zrb[0 : 32 * (len(HGROUPS[gi]) - 1) + 1, :],
                                      zr[0 : 32 * (len(HGROUPS[gi]) - 1) + 1, :])
                zbcs = []
                for k in range(len(HGROUPS[gi])):
                    zbc = sm.tile([DA, 512], bf16, tag="zbc", bufs=8, name="zbc")
                    src = zrb[32 * k : 32 * k + 1, :]
                    # partition-broadcast via zero-stride middle dim: the DMA
                    # reads the same 512-col line DA times
                    src_b = bass.AP(
                        src.tensor, src.offset, [[512, 1], [0, DA], [1, 512]]
                    )
                    nc.sync.dma_start(out=zbc, in_=src_b)
                    zbcs.append(zbc)
                norm_st[gi] = zbcs

            def norm_b(gi):
                """Normalize + repack into ctp for group gi."""
                zbcs = norm_st.pop(gi)
                for k, hh in enumerate(HGROUPS[gi]):
                    nc.vector.tensor_mul(ct[:, hh, :], hus.pop(hh), zbcs[k])
                    j0 = DA * hh
                    pl, off = j0 // 128, j0 % 128
                    l1 = min(128 - off, DA)
                    nc.gpsimd.dma_start(
                        out=ctp[off : off + l1, pl, :], in_=ct[0:l1, hh, :]
                    )
                    if l1 < DA:
                        nc.sync.dma_start(
                            out=ctp[0 : DA - l1, pl + 1, :], in_=ct[l1:DA, hh, :]
                        )

            # --- head phase, software pipelined: PE runs scores(h), qq(h+1),
            # R(h-2); normalization trails by group, spread over two
            # iterations (reciprocal is slow on DVE) and issued after the qq
            # cast so it never delays the next head's scores ---
            issue_qq(0)
            a_due, b_due = [], []
            for h in range(H):
                issue_scores(h)
                if h + 1 < H:
                    issue_qq(h + 1)
                if b_due:
                    norm_b(b_due.pop(0))
                if a_due:
                    gi = a_due.pop(0)
                    norm_a(gi)
                    b_due.append(gi)
                if h >= 2:
                    hh = h - 2
                    issue_r(hh)
                    gi = GOF[hh][0]
                    if hh == HGROUPS[gi][-1] and hh <= 11:
                        a_due.append(gi)

            # --- output projection: out[q, o] = sum_j ctp[j, q] g[j, o].
            # First two column groups start on planes 0..6 (ready before the
            # last norm group) to hide the final normalization chain. ---
            groups = [
                (qt, o0, o1)
                for qt in range(SH // 128)
                for (o0, o1) in [(0, 512), (512, 1024), (1024, 1280)]
            ]
            ops = {}

            def issue_gproj(gi, j_lo, j_hi, tag="sc"):
                qt, o0, o1 = groups[gi]
                if gi not in ops:
                    ops[gi] = ps.tile([128, 512], f32, tag=tag, bufs=2, name="op")
                op = ops[gi]
                for j in range(j_lo, j_hi):
                    kh = 128 if j < NPL - 1 else NJ - 128 * (NPL - 1)
                    nc.tensor.matmul(
                        op[:, 0 : o1 - o0],
                        ctp[0:kh, j, qt * 128 : qt * 128 + 128],
                        g_sb[0:kh, j, o0:o1],
                        start=(j == 0),
                        stop=(j == NPL - 1),
                    )
                if j_hi == NPL:
                    o_sb = sm.tile([128, 512], f32, tag="osb", bufs=2, name="o_sb")
                    nc.vector.tensor_copy(o_sb[:, 0 : o1 - o0], op[:, 0 : o1 - o0])
                    nc.gpsimd.dma_start(
                        out=out_d[qt * 128 : qt * 128 + 128, o0:o1],
                        in_=o_sb[:, 0 : o1 - o0],
                    )
                    del ops[gi]

            # plane deps: 0..7 <- heads <= 13; 8 <- head 14; 9,10 <- head 15.
            # Four concurrent accumulators (2 'sc' + 2 freed 'qr' slots) give
            # the PE cover work while the last norm chains complete.
            norm_a(3)
            norm_b(3)
            issue_r(14)
            norm_a(4)
            issue_gproj(0, 0, 8, "sc")
            issue_r(15)
            norm_a(5)
            norm_b(4)
            issue_gproj(1, 0, 8, "sc")
            norm_b(5)
            issue_gproj(2, 0, 8, "qr")
            issue_gproj(3, 0, 8, "qr")
            for gi in range(4):
                issue_gproj(gi, 8, 9)
            for gi in range(4):
                issue_gproj(gi, 9, NPL)
            for gi in range(4, len(groups)):
                issue_gproj(gi, 0, NPL)

    if legalize:
        _legalize_waits(nc, mybir)
    return nc


def _host_prep(Wq, bq, Wk, bk, Wv, bv, Wo, bo):
    """Weight-side host prep (shared by all cores)."""
    bf = ml_dtypes.bfloat16
    Wq, bq = np.asarray(Wq, np.float32), np.asarray(bq, np.float32)
    Wk, bk = np.asarray(Wk, np.float32), np.asarray(bk, np.float32)
    Wv, bv = np.asarray(Wv, np.float32), np.asarray(bv, np.float32)
    Wo, bo = np.asarray(Wo, np.float32), np.asarray(bo, np.float32)

    # mt[d1, h, d2] = (Wq_aug @ Wk_aug^T)[d1, d2], *_aug = [W^T; b] (81, 80)
    wq_aug = np.concatenate([Wq.transpose(0, 2, 1), bq[:, None, :]], 1)  # [H,81,80]
    wk_aug = np.concatenate([Wk.transpose(0, 2, 1), bk[:, None, :]], 1)
    mt = np.einsum("hde,hfe->dhf", wq_aug, wk_aug)  # [81, H, 81]
    mt = np.ascontiguousarray(mt).astype(bf)

    # G_h[d, o] = sum_e Wv_aug[d, e] Wo[o, 80h+e]; row d=0 is the bias row
    # (ones col of xs), bo folded into head 0's row 0.
    wv_aug = np.concatenate([bv[:, None, :], Wv.transpose(0, 2, 1)], 1)  # [H,81,80]
    wo_blocks = Wo.reshape(D, H, DK).transpose(1, 2, 0)  # [H, 80, D]
    g_flat = np.einsum("hde,heo->hdo", wv_aug, wo_blocks).reshape(NJ, D)
    g_flat[0] += bo
    g_pad = np.concatenate([g_flat, np.zeros((128 * NPL - NJ, D), np.float32)])
    g = np.ascontiguousarray(
        g_pad.reshape(NPL, 128, D).transpose(1, 0, 2)
    ).astype(bf)
    return mt, g


def _host_x(src_b, qlo):
    """Per-core activation prep: shuffle channels, roll queries to front,
    emit d-major (xh, ones row last) and s-major (xs, ones col first)."""
    bf = ml_dtypes.bfloat16
    sh = np.asarray(src_b, np.float32).reshape(S, G, D // G)
    sh = sh.transpose(0, 2, 1).reshape(S, D)  # channel shuffle
    xr = np.roll(sh, -qlo, axis=0)
    xh = np.concatenate(
        [xr.reshape(S, H, DK).transpose(2, 1, 0), np.ones((1, H, S), np.float32)]
    )  # [81, H, S]
    xs = np.concatenate(
        [
            np.ones((128, NT, H, 1), np.float32),
            xr.reshape(NT, 128, H, DK).transpose(1, 0, 2, 3),
        ],
        axis=3,
    )  # [128, NT, H, 81]
    return np.ascontiguousarray(xh).astype(bf), np.ascontiguousarray(xs).astype(bf)


def make_in_maps(inputs):
    src = np.asarray(inputs["src"], np.float32)
    mt, g = _host_prep(
        inputs["Wq"], inputs["bq"], inputs["Wk"], inputs["bk"],
        inputs["Wv"], inputs["bv"], inputs["Wo"], inputs["bo"],
    )
    in_maps = []
    for i in range(N_CORES):
        b, qlo = i // 2, (i % 2) * SH
        xh, xs = _host_x(src[b], qlo)
        in_maps.append({"xh": xh, "xs": xs, "mt": mt, "g": g})
    return in_maps


def kernel(**inputs):
    from concourse.bass_utils import run_bass_kernel_spmd

    if "nc" not in _BUILT:
        _BUILT["nc"] = _build()
    nc = _BUILT["nc"]

    in_maps = make_in_maps(inputs)
    res = run_bass_kernel_spmd(nc, in_maps, core_ids=list(range(N_CORES)))

    out = np.empty((B, S, D), np.float32)
    for i in range(N_CORES):
        b, qlo = i // 2, (i % 2) * SH
        out[b, qlo : qlo + SH] = res.results[i]["out"]
    return out


# revision 36
# speedup vs baseline: 1.2509x; 1.0300x over previous
"""Trainium2 Bass kernel for nn_MultiHeadAttention_60816736911814.

Reference semantics (all derived from `src`; `k`/`v` args ignored):
  x  = channel_shuffle(src)          # [B,S,G,C]->[B,S,C,G] flatten, G=5
  xh = split_heads(x)                # [B,H,S,dk], H=16, dk=80
  q/k/v = per-head Linear(dk,dk)     # weights [H,dk,dk] + bias
  attn  = softmax(q k^T / sqrt(dk)) v
  out   = concat(attn) @ Wo^T + bo   # Wo [D,D], D=1280

Sharding (8 cores, no collectives): core i handles batch b=i//2 and query
rows [512*(i%2), +512). Each core gets src[b] ROLLED so its query rows are
rows 0..511 (key order is irrelevant to softmax+sum); all cores run an
identical program and the full output is a pure concatenation.

Device-side algebraic restructuring (all folds are host-side, layout-only
or tiny weight-matrix products):
 - K projection eliminated: scores = x~_k^T Mt x~_q with Mt = Wq_aug @
   Wk_aug^T per head (x~ = [x;1] handles both biases). One projection
   (qq = Mt^T x~_q) replaces q AND k projections.
 - V projection and output projection fused: out = sum_h Rn_h^T G_h where
   R_h = XS_h P_h ([1+dk, SH], row 0 = Z = softmax denominator via the
   ones-column of XS), Rn = R * bcast(1/Z), and G_h = Wv_aug_h @
   Wo_h_block^T (+ bo folded into head 0's row 0, whose Rn value is 1).
 - Channel shuffle / head split / transposes are folded into the host-side
   layouts of xh (d-major) and xs (s-major): no on-device transposes.

Engine schedule: ACT does only the 48 exp instructions (scores land in
2/3-bank fused PSUM tiles so each Exp covers 1536/1024 columns); casts run
on GpSimd, normalization mul + reciprocal on DVE, 1/Z partition-broadcast
via SBUF->SBUF DMA. The PE stream is software-pipelined (scores(h) ->
qq(h+1) -> R(h-1)) so the Tensor engine stays continuously busy (max
p-state clock); qq and R share one rotating 2-slot PSUM tag so everything
fits the 8 PSUM banks alongside the 6 score banks.
"""

import numpy as np
import ml_dtypes

B, S, D = 4, 1024, 1280
H, DK, G = 16, 80, 5
N_CORES = 8
SH = S // 2  # 512 query rows per core
SCALE = 1.0 / float(np.sqrt(DK))
NT = S // 128  # 8 k-tiles
DA = DK + 1  # 81: augmented channel dim
NJ = H * DA  # 1296 packed (h,d) rows
NPL = (NJ + 127) // 128  # 11 planes
# per-head score fusion: k-tiles per fused PSUM tile / exp instruction
FUSE = [3, 3, 2]

_BUILT = {}


def _legalize_waits(nc, mybir):
    """This walrus build allows 1 sync-wait per instruction (2 on
    EventSemaphore). Tile can emit more; split overflow waits onto
    injected same-engine NoOp carriers placed just before the
    instruction (engines run their stream in order -> AND semantics)."""
    n_fix = 0
    for f in nc.m.functions:
        for blk in f.blocks:
            out = []
            changed = False
            for inst in blk.instructions:
                cap = 2 if type(inst).__name__ == "InstEventSemaphore" else 1
                si = inst.sync_info
                if si is not None and si.on_wait and len(si.on_wait) > cap:
                    waits = list(si.on_wait)
                    for w in waits[:-cap]:
                        nop = mybir.InstNoOp(name=f"I-waitfix-{n_fix}")
                        n_fix += 1
                        nop.engine = inst.engine
                        nop.sync_info = mybir.SyncInfo(on_wait=[w], on_update=[])
                        out.append(nop)
                    inst.sync_info = mybir.SyncInfo(
                        on_wait=waits[-cap:], on_update=list(si.on_update)
                    )
                    changed = True
                out.append(inst)
            if changed:
                try:
                    blk.instructions = out
                except Exception:
                    blk.instructions.clear()
                    blk.instructions.extend(out)
    return n_fix


def _ldw_peephole(nc):
    """Drop redundant weight reloads: a matmul whose stationary operand is
    byte-identical to the immediately preceding PE matmul's keeps the loaded
    weights (ldweights=False). Only Gproj's 3-way ocut reuse and the preheat
    chain match."""
    n = 0
    for f in nc.m.functions:
        for blk in f.blocks:
            prev = None
            for inst in blk.instructions:
                if type(inst).__name__ != "InstMatmult":
                    continue
                w = inst.ins[1]
                key = (
                    getattr(w, "memref", None),
                    getattr(w, "offset", None),
                    str(getattr(w, "ap", None)),
                    str(inst.is_transpose),
                    str(inst.perf_mode),
                )
                if prev == key:
                    inst.ldweights = False
                    n += 1
                prev = key
    return n


def _build(legalize=True):
    import os as _os
    import concourse.bass as bass
    import concourse.mybir as mybir
    import concourse.tile as tile

    _fuse = FUSE

    f32 = mybir.dt.float32
    bf16 = mybir.dt.bfloat16

    nc = bass.Bass(trn_type="TRN2", target_bir_lowering=False, debug=False)

    # xh[d, h, s]: d-major shuffled x, row 80 = ones
    xh_d = nc.dram_tensor("xh", [DA, H, S], bf16, kind="ExternalInput").ap()
    # xs[p, t, h, j]: s-major shuffled x (k = t*128+p), col j=0 = ones
    xs_d = nc.dram_tensor("xs", [128, NT, H, DA], bf16, kind="ExternalInput").ap()
    # mt[d1, h, d2] = (Wq_aug @ Wk_aug^T)[d1, d2] per head
    mt_d = nc.dram_tensor("mt", [DA, H, DA], bf16, kind="ExternalInput").ap()
    # g[p, pl, o]: packed rows j = 81h + d of G_h = Wv_aug @ Wo_h^T (+bo)
    g_d = nc.dram_tensor("g", [128, NPL, D], bf16, kind="ExternalInput").ap()
    out_d = nc.dram_tensor("out", [SH, D], f32, kind="ExternalOutput").ap()

    with tile.TileContext(nc) as tc:
        with (
            tc.tile_pool(name="const", bufs=1) as const,
            tc.tile_pool(name="big", bufs=1) as big,
            tc.tile_pool(name="et", bufs=6) as etp,
            tc.tile_pool(name="sm", bufs=2) as sm,
            tc.tile_pool(name="ps", bufs=2, space="PSUM") as ps,
        ):
            on2 = const.tile([128, 512], bf16)
            nc.gpsimd.memset(on2, 1.0)

            mt_sb = big.tile([DA, H, DA], bf16)
            xh_sb = big.tile([DA, H, S], bf16)
            xs_sb = big.tile([128, NT, H, DA], bf16)
            g_sb = big.tile([128, NPL, D], bf16)
            ct = big.tile([DA, H, SH], bf16)  # normalized heads, h-major
            ctp = big.tile([128, NPL, SH], bf16)  # packed rows j = 81h+d

            # --- input DMA: ACT issues none (DMA transfer time occupies the
            # issuing engine, and ACT paces the head phase) ---
            nc.sync.dma_start(out=mt_sb, in_=mt_d)
            nc.sync.dma_start(out=xh_sb[:, 0:2, :], in_=xh_d[:, 0:2, :])
            nc.sync.dma_start(out=xh_sb[:, 2:4, :], in_=xh_d[:, 2:4, :])
            nc.gpsimd.dma_start(out=xs_sb[:, 0:4, :, :], in_=xs_d[:, 0:4, :, :])
            nc.sync.dma_start(out=xs_sb[:, 4:8, :, :], in_=xs_d[:, 4:8, :, :])
            nc.sync.dma_start(out=xh_sb[:, 4:8, :], in_=xh_d[:, 4:8, :])
            nc.gpsimd.dma_start(out=xh_sb[:, 8:16, :], in_=xh_d[:, 8:16, :])
            nc.sync.dma_start(out=g_sb[:, 0:6, :], in_=g_d[:, 0:6, :])
            nc.gpsimd.dma_start(out=g_sb[:, 6:11, :], in_=g_d[:, 6:11, :])

            # --- PE p-state preheat: dummy matmuls so the clock is ramped
            # when real work starts ---
            for i in range(7):
                ph = ps.tile([DA, 512], f32, tag="qr", bufs=2, name="ph")
                nc.tensor.matmul(
                    ph[0:1, :], on2[0:1, 0:1], on2[0:1, :], start=True, stop=True
                )

            qq_sbs = {}
            ets = {}
            r_pss = {}
            hus = {}
            # head groups for softmax normalization; the last groups are
            # small so the final normalization chains are short
            HGROUPS = [
                (0, 1, 2, 3), (4, 5, 6, 7), (8, 9, 10, 11), (12, 13), (14,), (15,),
            ]
            GOF = {h: (gi, k) for gi, g in enumerate(HGROUPS) for k, h in enumerate(g)}
            zgs = {}
            norm_st = {}

            def issue_qq(h):
                qq_ps = ps.tile([DA, 512], f32, tag="qr", bufs=2, name="qq_ps")
                nc.tensor.matmul(
                    qq_ps, mt_sb[:, h, :], xh_sb[:, h, 0:SH], start=True, stop=True
                )
                qq_sb = sm.tile([DA, 512], bf16, tag="qq_sb", bufs=2, name="qq_sb")
                nc.vector.tensor_copy(qq_sb, qq_ps)
                qq_sbs[h] = qq_sb

            def issue_scores(h):
                qq_sb = qq_sbs.pop(h)
                ets[h] = []
                kt = 0
                for nf in _fuse:
                    sc = ps.tile([128, nf * 512], f32, tag="sc", bufs=2, name="sc")
                    for u in range(nf):
                        nc.tensor.matmul(
                            sc[:, u * 512 : u * 512 + 512],
                            xh_sb[:, h, (kt + u) * 128 : (kt + u) * 128 + 128],
                            qq_sb,
                            start=True,
                            stop=True,
                        )
                    kt += nf
                    et = etp.tile(
                        [128, nf * 512], bf16, tag="et", name="et",
                        bufs=3 * len(_fuse),
                    )
                    nc.scalar.activation(
                        et[:, 0 : nf * 512],
                        sc[:, 0 : nf * 512],
                        mybir.ActivationFunctionType.Exp,
                        scale=SCALE,
                    )
                    ets[h].append(et)

            def issue_r(h):
                r_ps = ps.tile([DA, 512], f32, tag="qr", bufs=2, name="r_ps")
                kt = 0
                for f, nf in enumerate(_fuse):
                    for u in range(nf):
                        nc.tensor.matmul(
                            r_ps,
                            xs_sb[:, kt + u, h, :],
                            ets[h][f][:, u * 512 : u * 512 + 512],
                            start=(kt + u == 0),
                            stop=(kt + u == NT - 1),
                        )
                    kt += nf
                del ets[h]
                gi, k = GOF[h]
                if len(HGROUPS[gi]) > 1:
                    if k == 0:
                        zgs[gi] = sm.tile(
                            [128, 512], f32, tag="zg", bufs=2, name="zg"
                        )
                        nc.gpsimd.memset(zgs[gi], 1.0)
                    nc.vector.tensor_copy(
                        zgs[gi][32 * k : 32 * k + 1, :], r_ps[0:1, :]
                    )
                else:
                    r_pss[h] = r_ps  # single-head group: recip reads PSUM row
                hu = sm.tile([DA, 512], bf16, tag="hu", bufs=8, name="hu")
                nc.vector.tensor_copy(hu, r_ps)
                hus[h] = hu

            def norm_a(gi):
                """Reciprocal of Z + broadcast DMAs for group gi."""
                zr = sm.tile([128, 512], f32, tag="zr", bufs=2, name="zr")
                if len(HGROUPS[gi]) > 1:
                    nc.vector.reciprocal(zr, zgs.pop(gi))
                else:
                    h = HGROUPS[gi][0]
                    nc.vector.reciprocal(zr[0:1, :], r_pss.pop(h)[0:1, :])
                zrb = sm.tile([128, 512], bf16, tag="zrb", bufs=2, name="zrb")
                nc.vector.tensor_copy(zrb[0 : 32 * (len(HGROUPS[gi]) - 1) + 1, :],
                                      zr[0 : 32 * (len(HGROUPS[gi]) - 1) + 1, :])
                zbcs = []
                for k in range(len(HGROUPS[gi])):
                    zbc = sm.tile([DA, 512], bf16, tag="zbc", bufs=8, name="zbc")
                    # partition-broadcast via zero-stride middle dim: the DMA
                    # reads the same line DA times; split across both DMA
                    # queues (the 81x re-read is bandwidth-bound)
                    for eng, c0, c1 in (
                        (nc.sync, 0, 256),
                        (nc.gpsimd, 256, 512),
                    ):
                        src = zrb[32 * k : 32 * k + 1, c0:c1]
                        src_b = bass.AP(
                            src.tensor, src.offset, [[512, 1], [0, DA], [1, c1 - c0]]
                        )
                        eng.dma_start(out=zbc[:, c0:c1], in_=src_b)
                    zbcs.append(zbc)
                norm_st[gi] = zbcs

            def norm_b(gi):
                """Normalize (on Pool: all-SBUF, keeps DVE clear) + repack."""
                zbcs = norm_st.pop(gi)
                for k, hh in enumerate(HGROUPS[gi]):
                    nc.gpsimd.tensor_mul(ct[:, hh, :], hus.pop(hh), zbcs[k])
                    j0 = DA * hh
                    pl, off = j0 // 128, j0 % 128
                    l1 = min(128 - off, DA)
                    nc.gpsimd.dma_start(
                        out=ctp[off : off + l1, pl, :], in_=ct[0:l1, hh, :]
                    )
                    if l1 < DA:
                        nc.sync.dma_start(
                            out=ctp[0 : DA - l1, pl + 1, :], in_=ct[l1:DA, hh, :]
                        )

            # --- head phase, software pipelined: PE runs scores(h), qq(h+1),
            # R(h-2); normalization trails by group, spread over two
            # iterations (reciprocal is slow on DVE) and issued after the qq
            # cast so it never delays the next head's scores ---
            issue_qq(0)
            a_due, b_due = [], []
            for h in range(H):
                issue_scores(h)
                if h + 1 < H:
                    issue_qq(h + 1)
                if b_due:
                    norm_b(b_due.pop(0))
                if a_due:
                    gi = a_due.pop(0)
                    norm_a(gi)
                    b_due.append(gi)
                if h >= 2:
                    hh = h - 2
                    issue_r(hh)
                    gi = GOF[hh][0]
                    if hh == HGROUPS[gi][-1] and hh <= 11:
                        a_due.append(gi)

            # --- output projection: out[q, o] = sum_j ctp[j, q] g[j, o].
            # Chunk-outer per q-tile: the three output-column accumulators
            # share each ctp stationary load (the ldweights peephole drops
            # the redundant reloads). ---
            OCUTS = [(0, 512), (512, 1024), (1024, 1280)]
            qt_ops = {}

            def gproj_qt(qt, j_lo, j_hi, tags=("sc", "sc", "qr")):
                if qt not in qt_ops:
                    qt_ops[qt] = [
                        ps.tile([128, 512], f32, tag=tags[c], bufs=2, name="op")
                        for c in range(3)
                    ]
                for j in range(j_lo, j_hi):
                    kh = 128 if j < NPL - 1 else NJ - 128 * (NPL - 1)
                    for c, (o0, o1) in enumerate(OCUTS):
                        nc.tensor.matmul(
                            qt_ops[qt][c][:, 0 : o1 - o0],
                            ctp[0:kh, j, qt * 128 : qt * 128 + 128],
                            g_sb[0:kh, j, o0:o1],
                            start=(j == 0),
                            stop=(j == NPL - 1),
                        )
                if j_hi == NPL:
                    for c, (o0, o1) in enumerate(OCUTS):
                        o_sb = sm.tile(
                            [128, 512], f32, tag="osb", bufs=2, name="o_sb"
                        )
                        nc.vector.tensor_copy(
                            o_sb[:, 0 : o1 - o0], qt_ops[qt][c][:, 0 : o1 - o0]
                        )
                        nc.gpsimd.dma_start(
                            out=out_d[qt * 128 : qt * 128 + 128, o0:o1],
                            in_=o_sb[:, 0 : o1 - o0],
                        )
                    del qt_ops[qt]

            # plane deps: 0..7 <- heads <= 13; 8 <- head 14; 9,10 <- head 15.
            # qt0's accumulators (2 'sc' + 1 'qr' slot) provide PE cover work
            # on planes 0..7 while the last norm chains complete.
            norm_a(3)
            norm_b(3)
            issue_r(14)
            norm_a(4)
            gproj_qt(0, 0, 8)
            issue_r(15)
            norm_a(5)
            norm_b(4)
            gproj_qt(0, 8, 9)
            norm_b(5)
            gproj_qt(0, 9, NPL)
            for qt in range(1, SH // 128):
                gproj_qt(qt, 0, NPL)

    _ldw_peephole(nc)
    if legalize:
        _legalize_waits(nc, mybir)
    return nc


def _host_prep(Wq, bq, Wk, bk, Wv, bv, Wo, bo):
    """Weight-side host prep (shared by all cores)."""
    bf = ml_dtypes.bfloat16
    Wq, bq = np.asarray(Wq, np.float32), np.asarray(bq, np.float32)
    Wk, bk = np.asarray(Wk, np.float32), np.asarray(bk, np.float32)
    Wv, bv = np.asarray(Wv, np.float32), np.asarray(bv, np.float32)
    Wo, bo = np.asarray(Wo, np.float32), np.asarray(bo, np.float32)

    # mt[d1, h, d2] = (Wq_aug @ Wk_aug^T)[d1, d2], *_aug = [W^T; b] (81, 80)
    wq_aug = np.concatenate([Wq.transpose(0, 2, 1), bq[:, None, :]], 1)  # [H,81,80]
    wk_aug = np.concatenate([Wk.transpose(0, 2, 1), bk[:, None, :]], 1)
    mt = np.einsum("hde,hfe->dhf", wq_aug, wk_aug)  # [81, H, 81]
    mt = np.ascontiguousarray(mt).astype(bf)

    # G_h[d, o] = sum_e Wv_aug[d, e] Wo[o, 80h+e]; row d=0 is the bias row
    # (ones col of xs), bo folded into head 0's row 0.
    wv_aug = np.concatenate([bv[:, None, :], Wv.transpose(0, 2, 1)], 1)  # [H,81,80]
    wo_blocks = Wo.reshape(D, H, DK).transpose(1, 2, 0)  # [H, 80, D]
    g_flat = np.einsum("hde,heo->hdo", wv_aug, wo_blocks).reshape(NJ, D)
    g_flat[0] += bo
    g_pad = np.concatenate([g_flat, np.zeros((128 * NPL - NJ, D), np.float32)])
    g = np.ascontiguousarray(
        g_pad.reshape(NPL, 128, D).transpose(1, 0, 2)
    ).astype(bf)
    return mt, g


def _host_x(src_b, qlo):
    """Per-core activation prep: shuffle channels, roll queries to front,
    emit d-major (xh, ones row last) and s-major (xs, ones col first)."""
    bf = ml_dtypes.bfloat16
    sh = np.asarray(src_b, np.float32).reshape(S, G, D // G)
    sh = sh.transpose(0, 2, 1).reshape(S, D)  # channel shuffle
    xr = np.roll(sh, -qlo, axis=0)
    xh = np.concatenate(
        [xr.reshape(S, H, DK).transpose(2, 1, 0), np.ones((1, H, S), np.float32)]
    )  # [81, H, S]
    xs = np.concatenate(
        [
            np.ones((128, NT, H, 1), np.float32),
            xr.reshape(NT, 128, H, DK).transpose(1, 0, 2, 3),
        ],
        axis=3,
    )  # [128, NT, H, 81]
    return np.ascontiguousarray(xh).astype(bf), np.ascontiguousarray(xs).astype(bf)


def make_in_maps(inputs):
    src = np.asarray(inputs["src"], np.float32)
    mt, g = _host_prep(
        inputs["Wq"], inputs["bq"], inputs["Wk"], inputs["bk"],
        inputs["Wv"], inputs["bv"], inputs["Wo"], inputs["bo"],
    )
    in_maps = []
    for i in range(N_CORES):
        b, qlo = i // 2, (i % 2) * SH
        xh, xs = _host_x(src[b], qlo)
        in_maps.append({"xh": xh, "xs": xs, "mt": mt, "g": g})
    return in_maps


def kernel(**inputs):
    from concourse.bass_utils import run_bass_kernel_spmd

    if "nc" not in _BUILT:
        _BUILT["nc"] = _build()
    nc = _BUILT["nc"]

    in_maps = make_in_maps(inputs)
    res = run_bass_kernel_spmd(nc, in_maps, core_ids=list(range(N_CORES)))

    out = np.empty((B, S, D), np.float32)
    for i in range(N_CORES):
        b, qlo = i // 2, (i % 2) * SH
        out[b, qlo : qlo + SH] = res.results[i]["out"]
    return out


# revision 43
# speedup vs baseline: 1.3024x; 1.0412x over previous
"""Trainium2 Bass kernel for nn_MultiHeadAttention_60816736911814.

Reference semantics (all derived from `src`; `k`/`v` args ignored):
  x  = channel_shuffle(src)          # [B,S,G,C]->[B,S,C,G] flatten, G=5
  xh = split_heads(x)                # [B,H,S,dk], H=16, dk=80
  q/k/v = per-head Linear(dk,dk)     # weights [H,dk,dk] + bias
  attn  = softmax(q k^T / sqrt(dk)) v
  out   = concat(attn) @ Wo^T + bo   # Wo [D,D], D=1280

Sharding (8 cores, no collectives): core i handles batch b=i//2 and query
rows [512*(i%2), +512). Each core gets src[b] ROLLED so its query rows are
rows 0..511 (key order is irrelevant to softmax+sum); all cores run an
identical program and the full output is a pure concatenation.

Device-side algebraic restructuring (all folds are host-side, layout-only
or tiny weight-matrix products):
 - K projection eliminated: scores = x~_k^T Mt x~_q with Mt = Wq_aug @
   Wk_aug^T per head (x~ = [x;1] handles both biases). One projection
   (qq = Mt^T x~_q) replaces q AND k projections.
 - V projection and output projection fused: out = sum_h Rn_h^T G_h where
   R_h = XS_h P_h ([1+dk, SH], row 0 = Z = softmax denominator via the
   ones-column of XS), Rn = R * bcast(1/Z), and G_h = Wv_aug_h @
   Wo_h_block^T (+ bo folded into head 0's row 0, whose Rn value is 1).
 - Channel shuffle / head split / transposes are folded into the host-side
   layouts of xh (d-major) and xs (s-major): no on-device transposes.

Engine schedule: ACT does only the 48 exp instructions (scores land in
2/3-bank fused PSUM tiles so each Exp covers 1536/1024 columns); casts run
on GpSimd, normalization mul + reciprocal on DVE, 1/Z partition-broadcast
via SBUF->SBUF DMA. The PE stream is software-pipelined (scores(h) ->
qq(h+1) -> R(h-1)) so the Tensor engine stays continuously busy (max
p-state clock); qq and R share one rotating 2-slot PSUM tag so everything
fits the 8 PSUM banks alongside the 6 score banks.
"""

import numpy as np
import ml_dtypes

B, S, D = 4, 1024, 1280
H, DK, G = 16, 80, 5
N_CORES = 8
SH = S // 2  # 512 query rows per core
SCALE = 1.0 / float(np.sqrt(DK))
NT = S // 128  # 8 k-tiles
DA = DK + 1  # 81: augmented channel dim
NJ = H * DA  # 1296 packed (h,d) rows
NPL = (NJ + 127) // 128  # 11 planes
# per-head score fusion: k-tiles per fused PSUM tile / exp instruction
FUSE = [3, 3, 2]

_BUILT = {}


def _legalize_waits(nc, mybir):
    """This walrus build allows 1 sync-wait per instruction (2 on
    EventSemaphore). Tile can emit more; split overflow waits onto
    injected same-engine NoOp carriers placed just before the
    instruction (engines run their stream in order -> AND semantics)."""
    n_fix = 0
    for f in nc.m.functions:
        for blk in f.blocks:
            out = []
            changed = False
            for inst in blk.instructions:
                cap = 2 if type(inst).__name__ == "InstEventSemaphore" else 1
                si = inst.sync_info
                if si is not None and si.on_wait and len(si.on_wait) > cap:
                    waits = list(si.on_wait)
                    for w in waits[:-cap]:
                        nop = mybir.InstNoOp(name=f"I-waitfix-{n_fix}")
                        n_fix += 1
                        nop.engine = inst.engine
                        nop.sync_info = mybir.SyncInfo(on_wait=[w], on_update=[])
                        out.append(nop)
                    inst.sync_info = mybir.SyncInfo(
                        on_wait=waits[-cap:], on_update=list(si.on_update)
                    )
                    changed = True
                out.append(inst)
            if changed:
                try:
                    blk.instructions = out
                except Exception:
                    blk.instructions.clear()
                    blk.instructions.extend(out)
    return n_fix


def _ldw_peephole(nc):
    """Drop redundant weight reloads: a matmul whose stationary operand is
    byte-identical to the immediately preceding PE matmul's keeps the loaded
    weights (ldweights=False). Only Gproj's 3-way ocut reuse and the preheat
    chain match."""
    n = 0
    for f in nc.m.functions:
        for blk in f.blocks:
            prev = None
            for inst in blk.instructions:
                if type(inst).__name__ != "InstMatmult":
                    continue
                w = inst.ins[1]
                key = (
                    getattr(w, "memref", None),
                    getattr(w, "offset", None),
                    str(getattr(w, "ap", None)),
                    str(inst.is_transpose),
                    str(inst.perf_mode),
                )
                if prev == key:
                    inst.ldweights = False
                    n += 1
                prev = key
    return n


def _build(legalize=True):
    import os as _os
    import concourse.bass as bass
    import concourse.mybir as mybir
    import concourse.tile as tile

    _fuse = FUSE

    f32 = mybir.dt.float32
    bf16 = mybir.dt.bfloat16

    nc = bass.Bass(trn_type="TRN2", target_bir_lowering=False, debug=False)

    # xh[d, h, s]: d-major shuffled x, row 80 = ones
    xh_d = nc.dram_tensor("xh", [DA, H, S], bf16, kind="ExternalInput").ap()
    # xs[p, h, t, j]: s-major shuffled x (k = t*128+p), col j=0 = ones
    xs_d = nc.dram_tensor("xs", [128, H, NT, DA], bf16, kind="ExternalInput").ap()
    # mt[d1, h, d2] = (Wq_aug @ Wk_aug^T)[d1, d2] per head
    mt_d = nc.dram_tensor("mt", [DA, H, DA], bf16, kind="ExternalInput").ap()
    # g[p, pl, o]: packed rows j = 81h + d of G_h = Wv_aug @ Wo_h^T (+bo)
    g_d = nc.dram_tensor("g", [128, NPL, D], bf16, kind="ExternalInput").ap()
    out_d = nc.dram_tensor("out", [SH, D], f32, kind="ExternalOutput").ap()

    with tile.TileContext(nc) as tc:
        with (
            tc.tile_pool(name="const", bufs=1) as const,
            tc.tile_pool(name="big", bufs=1) as big,
            tc.tile_pool(name="et", bufs=6) as etp,
            tc.tile_pool(name="sm", bufs=2) as sm,
            tc.tile_pool(name="ps", bufs=2, space="PSUM") as ps,
        ):
            on2 = const.tile([128, 512], bf16)
            nc.gpsimd.memset(on2, 1.0)

            mt_sb = big.tile([DA, H, DA], bf16)
            xh_sb = big.tile([DA, H, S], bf16)
            xs_sb = big.tile([128, H, NT, DA], bf16)
            g_sb = big.tile([128, NPL, D], bf16)
            ct = big.tile([DA, H, SH], bf16)  # normalized heads, h-major
            ctp = big.tile([128, NPL, SH], bf16)  # packed rows j = 81h+d

            # --- input DMA: demand-driven. Only what iterations 0..3 need is
            # loaded upfront; the rest is prefetched per-head inside the loop
            # so no single bulk transfer blocks startup. ACT issues no DMAs
            # (DMA transfer time occupies the issuing engine). ---
            nc.sync.dma_start(out=mt_sb, in_=mt_d)
            nc.sync.dma_start(out=xh_sb[:, 0:2, :], in_=xh_d[:, 0:2, :])
            nc.gpsimd.dma_start(out=xs_sb[:, 0:2, :, :], in_=xs_d[:, 0:2, :, :])
            nc.sync.dma_start(out=xh_sb[:, 2:4, :], in_=xh_d[:, 2:4, :])

            # --- PE p-state preheat: dummy matmuls so the clock is ramped
            # when real work starts ---
            for i in range(7):
                ph = ps.tile([DA, 512], f32, tag="qr", bufs=2, name="ph")
                nc.tensor.matmul(
                    ph[0:1, :], on2[0:1, 0:1], on2[0:1, :], start=True, stop=True
                )

            qq_sbs = {}
            ets = {}
            r_pss = {}
            hus = {}
            # head groups for softmax normalization; the last groups are
            # small so the final normalization chains are short
            HGROUPS = [
                (0, 1, 2, 3), (4, 5, 6, 7), (8, 9, 10, 11), (12, 13), (14,), (15,),
            ]
            GOF = {h: (gi, k) for gi, g in enumerate(HGROUPS) for k, h in enumerate(g)}
            zgs = {}
            norm_st = {}

            fetched = {("xh", 0), ("xh", 1), ("xh", 2), ("xh", 3),
                       ("xs", 0), ("xs", 1)}

            def prefetch(kind, h):
                if h > H - 1 or (kind, h) in fetched:
                    return
                fetched.add((kind, h))
                if kind == "xh":
                    nc.sync.dma_start(
                        out=xh_sb[:, h : h + 1, :], in_=xh_d[:, h : h + 1, :]
                    )
                else:
                    nc.gpsimd.dma_start(
                        out=xs_sb[:, h : h + 1, :, :], in_=xs_d[:, h : h + 1, :, :]
                    )

            def issue_qq(h):
                prefetch("xh", h + 3)
                prefetch("xs", h + 1)
                qq_ps = ps.tile([DA, 512], f32, tag="qr", bufs=2, name="qq_ps")
                nc.tensor.matmul(
                    qq_ps, mt_sb[:, h, :], xh_sb[:, h, 0:SH], start=True, stop=True
                )
                qq_sb = sm.tile([DA, 512], bf16, tag="qq_sb", bufs=2, name="qq_sb")
                nc.vector.tensor_copy(qq_sb, qq_ps)
                qq_sbs[h] = qq_sb

            def issue_scores(h):
                if 8 <= h <= 12:
                    j = 2 * (h - 8)
                    eng = nc.sync if h % 2 == 0 else nc.gpsimd
                    eng.dma_start(
                        out=g_sb[:, j : j + 2, :], in_=g_d[:, j : j + 2, :]
                    )
                elif h == 13:
                    nc.sync.dma_start(
                        out=g_sb[:, 10:11, :], in_=g_d[:, 10:11, :]
                    )
                qq_sb = qq_sbs.pop(h)
                ets[h] = []
                kt = 0
                for nf in _fuse:
                    sc = ps.tile([128, nf * 512], f32, tag="sc", bufs=2, name="sc")
                    for u in range(nf):
                        nc.tensor.matmul(
                            sc[:, u * 512 : u * 512 + 512],
                            xh_sb[:, h, (kt + u) * 128 : (kt + u) * 128 + 128],
                            qq_sb,
                            start=True,
                            stop=True,
                        )
                    kt += nf
                    et = etp.tile(
                        [128, nf * 512], bf16, tag="et", name="et",
                        bufs=3 * len(_fuse),
                    )
                    nc.scalar.activation(
                        et[:, 0 : nf * 512],
                        sc[:, 0 : nf * 512],
                        mybir.ActivationFunctionType.Exp,
                        scale=SCALE,
                    )
                    ets[h].append(et)

            def issue_r(h):
                r_ps = ps.tile([DA, 512], f32, tag="qr", bufs=2, name="r_ps")
                kt = 0
                for f, nf in enumerate(_fuse):
                    for u in range(nf):
                        nc.tensor.matmul(
                            r_ps,
                            xs_sb[:, h, kt + u, :],
                            ets[h][f][:, u * 512 : u * 512 + 512],
                            start=(kt + u == 0),
                            stop=(kt + u == NT - 1),
                        )
                    kt += nf
                del ets[h]
                gi, k = GOF[h]
                if len(HGROUPS[gi]) > 1:
                    if k == 0:
                        zgs[gi] = sm.tile(
                            [128, 512], f32, tag="zg", bufs=2, name="zg"
                        )
                        nc.gpsimd.memset(zgs[gi], 1.0)
                    nc.scalar.copy(
                        zgs[gi][32 * k : 32 * k + 1, :], r_ps[0:1, :]
                    )
                else:
                    r_pss[h] = r_ps  # single-head group: recip reads PSUM row
                hu = sm.tile([DA, 512], bf16, tag="hu", bufs=8, name="hu")
                nc.vector.tensor_copy(hu, r_ps)
                hus[h] = hu

            def norm_a(gi, nch=1):
                """Reciprocal of Z + broadcast DMAs for group gi. nch>1
                pipelines the chain in q-tile chunks (tail latency)."""
                nr = 32 * (len(HGROUPS[gi]) - 1) + 1
                zr = sm.tile([128, 512], f32, tag="zr", bufs=2, name="zr")
                zin = zgs.pop(gi) if len(HGROUPS[gi]) > 1 else r_pss.pop(
                    HGROUPS[gi][0]
                )
                w = 512 // nch
                for c in range(nch):
                    nc.vector.reciprocal(
                        zr[0:nr, c * w : c * w + w], zin[0:nr, c * w : c * w + w]
                    )
                zrb = sm.tile([128, 512], bf16, tag="zrb", bufs=2, name="zrb")
                for c in range(nch):
                    nc.vector.tensor_copy(
                        zrb[0:nr, c * w : c * w + w], zr[0:nr, c * w : c * w + w]
                    )
                zbcs = []
                for k in range(len(HGROUPS[gi])):
                    zbc = sm.tile([DA, 512], bf16, tag="zbc", bufs=8, name="zbc")
                    # partition-broadcast via zero-stride middle dim: the DMA
                    # reads the same line DA times; split across both DMA
                    # queues (the 81x re-read is bandwidth-bound)
                    h2 = 256 // nch
                    for c in range(nch):
                        for eng, c0 in ((nc.sync, 2 * c * h2), (nc.gpsimd, 2 * c * h2 + h2)):
                            src = zrb[32 * k : 32 * k + 1, c0 : c0 + h2]
                            src_b = bass.AP(
                                src.tensor, src.offset,
                                [[512, 1], [0, DA], [1, h2]],
                            )
                            eng.dma_start(out=zbc[:, c0 : c0 + h2], in_=src_b)
                    zbcs.append(zbc)
                norm_st[gi] = zbcs

            def norm_b(gi, nch=1):
                """Normalize + repack. In-phase groups multiply on Pool
                (all-SBUF, keeps DVE clear); tail groups on DVE chunked."""
                zbcs = norm_st.pop(gi)
                eng_mul = nc.vector.tensor_mul if gi >= 3 else nc.gpsimd.tensor_mul
                w = 512 // nch
                for k, hh in enumerate(HGROUPS[gi]):
                    hu = hus.pop(hh)
                    j0 = DA * hh
                    pl, off = j0 // 128, j0 % 128
                    l1 = min(128 - off, DA)
                    for c in range(nch):
                        s = slice(c * w, c * w + w)
                        eng_mul(ct[:, hh, s], hu[:, s], zbcs[k][:, s])
                        nc.gpsimd.dma_start(
                            out=ctp[off : off + l1, pl, s], in_=ct[0:l1, hh, s]
                        )
                        if l1 < DA:
                            nc.sync.dma_start(
                                out=ctp[0 : DA - l1, pl + 1, s],
                                in_=ct[l1:DA, hh, s],
                            )

            # --- head phase, software pipelined: PE runs scores(h), qq(h+1),
            # R(h-2); normalization trails by group, spread over two
            # iterations (reciprocal is slow on DVE) and issued after the qq
            # cast so it never delays the next head's scores ---
            issue_qq(0)
            a_due, b_due = [], []
            for h in range(H):
                issue_scores(h)
                if h + 1 < H:
                    issue_qq(h + 1)
                if b_due:
                    norm_b(b_due.pop(0))
                if a_due:
                    gi = a_due.pop(0)
                    norm_a(gi)
                    b_due.append(gi)
                if h >= 2:
                    hh = h - 2
                    issue_r(hh)
                    gi = GOF[hh][0]
                    if hh == HGROUPS[gi][-1] and hh <= 11:
                        a_due.append(gi)

            # --- output projection: out[q, o] = sum_j ctp[j, q] g[j, o].
            # Chunk-outer per q-tile: the three output-column accumulators
            # share each ctp stationary load (the ldweights peephole drops
            # the redundant reloads). ---
            OCUTS = [(0, 512), (512, 1024), (1024, 1280)]
            qt_ops = {}

            def gproj_qt(qt, j_lo, j_hi, tags=("sc", "sc", "qr")):
                if qt not in qt_ops:
                    qt_ops[qt] = [
                        ps.tile([128, 512], f32, tag=tags[c], bufs=2, name="op")
                        for c in range(3)
                    ]
                for j in range(j_lo, j_hi):
                    kh = 128 if j < NPL - 1 else NJ - 128 * (NPL - 1)
                    for c, (o0, o1) in enumerate(OCUTS):
                        nc.tensor.matmul(
                            qt_ops[qt][c][:, 0 : o1 - o0],
                            ctp[0:kh, j, qt * 128 : qt * 128 + 128],
                            g_sb[0:kh, j, o0:o1],
                            start=(j == 0),
                            stop=(j == NPL - 1),
                        )
                if j_hi == NPL:
                    for c, (o0, o1) in enumerate(OCUTS):
                        o_sb = sm.tile(
                            [128, 512], f32, tag="osb", bufs=2, name="o_sb"
                        )
                        nc.vector.tensor_copy(
                            o_sb[:, 0 : o1 - o0], qt_ops[qt][c][:, 0 : o1 - o0]
                        )
                        nc.gpsimd.dma_start(
                            out=out_d[qt * 128 : qt * 128 + 128, o0:o1],
                            in_=o_sb[:, 0 : o1 - o0],
                        )
                    del qt_ops[qt]

            # plane deps: 0..7 <- heads <= 13; 8 <- head 14; 9,10 <- head 15.
            # qt0's accumulators (2 'sc' + 1 'qr' slot) provide PE cover work
            # on planes 0..7 while the last norm chains complete.
            norm_a(3)
            norm_b(3)
            issue_r(14)
            norm_a(4, nch=4)
            gproj_qt(0, 0, 8)
            issue_r(15)
            norm_a(5, nch=4)
            norm_b(4, nch=4)
            gproj_qt(0, 8, 9)
            norm_b(5, nch=4)
            gproj_qt(0, 9, NPL)
            for qt in range(1, SH // 128):
                gproj_qt(qt, 0, NPL)

    _ldw_peephole(nc)
    if legalize:
        _legalize_waits(nc, mybir)
    return nc


def _host_prep(Wq, bq, Wk, bk, Wv, bv, Wo, bo):
    """Weight-side host prep (shared by all cores)."""
    bf = ml_dtypes.bfloat16
    Wq, bq = np.asarray(Wq, np.float32), np.asarray(bq, np.float32)
    Wk, bk = np.asarray(Wk, np.float32), np.asarray(bk, np.float32)
    Wv, bv = np.asarray(Wv, np.float32), np.asarray(bv, np.float32)
    Wo, bo = np.asarray(Wo, np.float32), np.asarray(bo, np.float32)

    # mt[d1, h, d2] = (Wq_aug @ Wk_aug^T)[d1, d2], *_aug = [W^T; b] (81, 80)
    wq_aug = np.concatenate([Wq.transpose(0, 2, 1), bq[:, None, :]], 1)  # [H,81,80]
    wk_aug = np.concatenate([Wk.transpose(0, 2, 1), bk[:, None, :]], 1)
    mt = np.einsum("hde,hfe->dhf", wq_aug, wk_aug)  # [81, H, 81]
    mt = np.ascontiguousarray(mt).astype(bf)

    # G_h[d, o] = sum_e Wv_aug[d, e] Wo[o, 80h+e]; row d=0 is the bias row
    # (ones col of xs), bo folded into head 0's row 0.
    wv_aug = np.concatenate([bv[:, None, :], Wv.transpose(0, 2, 1)], 1)  # [H,81,80]
    wo_blocks = Wo.reshape(D, H, DK).transpose(1, 2, 0)  # [H, 80, D]
    g_flat = np.einsum("hde,heo->hdo", wv_aug, wo_blocks).reshape(NJ, D)
    g_flat[0] += bo
    g_pad = np.concatenate([g_flat, np.zeros((128 * NPL - NJ, D), np.float32)])
    g = np.ascontiguousarray(
        g_pad.reshape(NPL, 128, D).transpose(1, 0, 2)
    ).astype(bf)
    return mt, g


def _host_x(src_b, qlo):
    """Per-core activation prep: shuffle channels, roll queries to front,
    emit d-major (xh, ones row last) and s-major (xs, ones col first)."""
    bf = ml_dtypes.bfloat16
    sh = np.asarray(src_b, np.float32).reshape(S, G, D // G)
    sh = sh.transpose(0, 2, 1).reshape(S, D)  # channel shuffle
    xr = np.roll(sh, -qlo, axis=0)
    xh = np.concatenate(
        [xr.reshape(S, H, DK).transpose(2, 1, 0), np.ones((1, H, S), np.float32)]
    )  # [81, H, S]
    xs = np.concatenate(
        [
            np.ones((128, H, NT, 1), np.float32),
            xr.reshape(NT, 128, H, DK).transpose(1, 2, 0, 3),
        ],
        axis=3,
    )  # [128, H, NT, 81]
    return np.ascontiguousarray(xh).astype(bf), np.ascontiguousarray(xs).astype(bf)


def make_in_maps(inputs):
    src = np.asarray(inputs["src"], np.float32)
    mt, g = _host_prep(
        inputs["Wq"], inputs["bq"], inputs["Wk"], inputs["bk"],
        inputs["Wv"], inputs["bv"], inputs["Wo"], inputs["bo"],
    )
    in_maps = []
    for i in range(N_CORES):
        b, qlo = i // 2, (i % 2) * SH
        xh, xs = _host_x(src[b], qlo)
        in_maps.append({"xh": xh, "xs": xs, "mt": mt, "g": g})
    return in_maps


def kernel(**inputs):
    from concourse.bass_utils import run_bass_kernel_spmd

    if "nc" not in _BUILT:
        _BUILT["nc"] = _build()
    nc = _BUILT["nc"]

    in_maps = make_in_maps(inputs)
    res = run_bass_kernel_spmd(nc, in_maps, core_ids=list(range(N_CORES)))

    out = np.empty((B, S, D), np.float32)
    for i in range(N_CORES):
        b, qlo = i // 2, (i % 2) * SH
        out[b, qlo : qlo + SH] = res.results[i]["out"]
    return out
